# revision 1
# baseline (speedup 1.0000x reference)
"""YOLO-detect head (1x1 conv + box decode) on 8 Trainium2 NeuronCores.

Data-parallel over batch: core b processes batch element b.

Per core, per level l (C channels, HW = ny*nx positions):
  p[hw, o] = sum_c x[c, hw] * w[o, c]      (o = a*89 + ch, a anchor, ch channel)
computed on the tensor engine as out = lhsT.T @ rhs with
  lhsT = x chunk  [K=128 channels, M<=128 hw]   (stationary, fp16)
  rhs  = w.T chunk [K=128 channels, N=267]      (moving, fp16)
so the PSUM result is already [hw, 267] — no on-chip transpose.

Decode:
  sigmoid(p) is computed as 0.5*tanh(0.5*p) + 0.5 so that the only ACT table
  set ever needed is exp_and_others (holds BOTH tanh and exp) -> exactly one
  ~2.7us ACT table load for the whole kernel instead of one per
  sigmoid<->exp alternation.
  xy   = sigmoid(p)*stride + grid*stride   (grid*stride precomputed on host)
  wh   = exp(p) * anchor
  rest = sigmoid(p)

DMA regime (what profiling showed matters):
  * The natural (25200, 89) output costs one 356-byte packet per row; instead
    each level is stored as (128, NA, R, 89) — partition p holds rows
    {t*128+p} of each anchor contiguously — and the host transposes back.
  * HWDGE descriptor generation serializes on the issuing sequencer at
    ~0.7us per dma_start, and a blocked store at the head of the FIFO stalls
    every later DMA. So: inputs are host-permuted so each level's x / w loads
    are 1-2 large fully-contiguous-per-partition DMAs on nc.sync, and stores
    go through nc.gpsimd (SWDGE, otherwise-idle queue) so their compute waits
    never block loads.

Inputs x/w are cast to fp16 on host (halves HBM load traffic vs fp32; fp16's
11-bit mantissa + fp32 accumulate keeps the error ~2e-4 of output scale).
"""

import numpy as np

import concourse.bacc as bacc
import concourse.mybir as mybir
import concourse.tile as tile
from concourse.bass_utils import run_bass_kernel_spmd

F32 = mybir.dt.float32
F16 = mybir.dt.float16
AF = mybir.ActivationFunctionType
ALU = mybir.AluOpType

NCORES = 8
NA = 3          # anchors per level
NO = 89         # channels per anchor (80 classes + 5 + 4)
NCOL = NA * NO  # 267
GROUP = 2       # full 128-row hw tiles per PSUM group (2 banks)

LEVELS = [
    dict(C=256,  W=80, HW=6400, stride=8.0,
         anchors=((10.0, 13.0), (16.0, 30.0), (33.0, 23.0))),
    dict(C=512,  W=40, HW=1600, stride=16.0,
         anchors=((30.0, 61.0), (62.0, 45.0), (59.0, 119.0))),
    dict(C=1024, W=20, HW=400,  stride=32.0,
         anchors=((116.0, 90.0), (156.0, 198.0), (373.0, 326.0))),
]


def _ntiles(HW):
    return (HW + 127) // 128


def _groups(HW):
    """[(t0, n_full_tiles, rows_per_tile)]; trailing partial tile is its own group."""
    full, rem = divmod(HW, 128)
    out = []
    t0 = 0
    while t0 < full:
        n = min(GROUP, full - t0)
        out.append((t0, n, 128))
        t0 += n
    if rem:
        out.append((full, 1, rem))
    return out


# levels processed smallest-first: the tiny level-2/1 matmuls start while the
# big level-0 x tensor is still streaming in, and they warm the PE clock gate
ORDER = (0, 1, 2)


def _store_chunks(nt):
    """1-3 store chunks per level (each one anchor-merged DMA)."""
    if nt <= 4 * GROUP:
        return [(0, nt)]
    chunks = []
    s = 0
    while s < nt:
        e = min(s + 4 * GROUP, nt)
        if nt - e <= GROUP:
            e = nt
        chunks.append((s, e - s))
        s = e
    return chunks


def _build_program(use_bias: bool):
    # Bacc (not raw Bass): its compile() runs move_matmul_waits_to_ldweights +
    # generate_event_semaphores, without which walrus rejects instructions
    # that carry more than one semaphore wait.
    nc = bacc.Bacc("TRN2", target_bir_lowering=False, debug=False)

    GSAM_COLS = sum(_ntiles(L["HW"]) * 12 for L in LEVELS)  # 804

    dram = {}
    for l, L in enumerate(LEVELS):
        KC = L["C"] // 128
        nt = _ntiles(L["HW"])
        # x / wt are host-permuted: row p, col (k*HW + w) = x[k*128+p, w]
        dram[f"x{l}"] = nc.dram_tensor(f"x{l}", (128, KC * L["HW"]), F16,
                                       kind="ExternalInput").ap()
        dram[f"wt{l}"] = nc.dram_tensor(f"wt{l}", (128, KC * NCOL), F16,
                                        kind="ExternalInput").ap()
        dram[f"y{l}"] = nc.dram_tensor(f"y{l}", (128, NA, nt, NO), F16,
                                       kind="ExternalOutput").ap()
        if use_bias:
            dram[f"b{l}"] = nc.dram_tensor(f"b{l}", (1, NCOL), F32,
                                           kind="ExternalInput").ap()
    dram["gsam"] = nc.dram_tensor("gsam", (128, GSAM_COLS), F16,
                                  kind="ExternalInput").ap()

    with tile.TileContext(nc) as tc:
        with tc.tile_pool(name="consts", bufs=1) as cpool, \
             tc.tile_pool(name="xbuf", bufs=1) as xpool, \
             tc.tile_pool(name="obuf", bufs=1) as opool, \
             tc.tile_pool(name="ps", bufs=4, space="PSUM") as pspool:

            ones_t = None
            if use_bias:
                ones_t = cpool.tile([1, 128], F16, tag="ones", name="ones")
                nc.vector.memset(ones_t[:, :], 1.0)

            # ---- Phase A: all loads (nc.sync ring carries loads only) ----
            lvl = {}
            for l in ORDER:
                L = LEVELS[l]
                C, HW = L["C"], L["HW"]
                KC = C // 128
                wt_t = cpool.tile([128, KC * NCOL], F16, tag=f"wt{l}",
                                  name=f"wt{l}sb")
                nc.sync.dma_start(out=wt_t[:, :], in_=dram[f"wt{l}"][:, :])

                xk = xpool.tile([128, KC * HW], F16, tag=f"x{l}", name=f"xk{l}")
                if l == 0:
                    # three column-piece DMAs so level-0 matmuls start earlier
                    xs = dram[f"x{l}"].rearrange("p (k w) -> p k w", k=KC)
                    xd = xk.rearrange("p (k w) -> p k w", k=KC)
                    for (c0, c1) in ((0, 2048), (2048, 4224), (4224, HW)):
                        nc.sync.dma_start(out=xd[:, :, c0:c1],
                                          in_=xs[:, :, c0:c1])
                else:
                    nc.sync.dma_start(out=xk[:, :], in_=dram[f"x{l}"][:, :])

                b_t = None
                if use_bias:
                    b_t = cpool.tile([1, NCOL], F16, tag=f"b{l}", name=f"bt{l}")
                    nc.gpsimd.dma_start(out=b_t[:, :], in_=dram[f"b{l}"][:, :])
                lvl[l] = dict(wt=wt_t, xk=xk, b_t=b_t)

                if l == ORDER[0]:
                    gsam_t = cpool.tile([128, GSAM_COLS], F16, tag="gsam",
                                        name="gsamsb")
                    nc.sync.dma_start(out=gsam_t[:, :], in_=dram["gsam"][:, :])

            off = 0
            for l, L in enumerate(LEVELS):
                nt = _ntiles(L["HW"])
                lvl[l]["gs"] = gsam_t[:, off:off + nt * 6].rearrange(
                    "p (t a c) -> p t a c", a=NA, c=2)
                off += nt * 6
                lvl[l]["am"] = gsam_t[:, off:off + nt * 6].rearrange(
                    "p (t a c) -> p t a c", a=NA, c=2)
                off += nt * 6

            # ---- Phase B: compute; stores via SWDGE (gpsimd) ----
            for l in ORDER:
                L = LEVELS[l]
                C, HW, stride = L["C"], L["HW"], L["stride"]
                KC = C // 128
                nt = _ntiles(HW)
                wt_t, xk, b_t = lvl[l]["wt"], lvl[l]["xk"], lvl[l]["b_t"]
                gs_t, am_t = lvl[l]["gs"], lvl[l]["am"]

                # whole level's decoded output stays resident, anchor-major so
                # each (partition, anchor) store run is contiguous; partition p
                # element (a, t, :) is output row hw = t*128+p of anchor a
                ot = opool.tile([128, NA, nt, NO], F16, tag=f"ot{l}", name=f"ot{l}")

                chunks = _store_chunks(nt)
                next_chunk = 0

                for (t0, ntl, m) in _groups(HW):
                    ps = pspool.tile([128, GROUP, 512], F32, tag="ps",
                                     name=f"ps{l}_{t0}")
                    psf = ps.rearrange("p g x -> p (g x)")
                    for i in range(ntl):
                        t = t0 + i
                        for kc in range(KC):
                            nc.tensor.matmul(
                                psf[0:m, i * 512:i * 512 + NCOL],
                                lhsT=xk[:, kc * HW + t * 128:kc * HW + t * 128 + m],
                                rhs=wt_t[:, kc * NCOL:(kc + 1) * NCOL],
                                start=(kc == 0),
                                stop=(kc == KC - 1 and not use_bias),
                            )
                        if use_bias:
                            nc.tensor.matmul(
                                psf[0:m, i * 512:i * 512 + NCOL],
                                lhsT=ones_t[:, 0:m],
                                rhs=b_t[:, :],
                                start=False,
                                stop=True,
                            )

                    og = ot[0:m, :, t0:t0 + ntl, :]  # (m, NA, ntl, 89)
                    # psum viewed anchor-major to match og's enumeration
                    ps_a = ps[0:m, 0:ntl, 0:NCOL].rearrange(
                        "p g (a c) -> p a g c", a=NA)
                    # t = tanh(0.5 * p); sigmoid(p) = 0.5*t + 0.5
                    nc.scalar.activation(og, ps_a, AF.Tanh, scale=0.5)
                    # merged (g c) innermost dim is even -> DVE 2x mode
                    ogf = og.rearrange("p a g c -> p a (g c)")
                    nc.vector.tensor_scalar(ogf, ogf, 1.0, 0.5, ALU.add, ALU.mult)
                    # wh: exp(p) (overwrites the sigmoid values on those cols)
                    nc.scalar.activation(og[:, :, :, 2:4], ps_a[:, :, :, 2:4],
                                         AF.Exp)
                    am_a = am_t[0:m, t0:t0 + ntl].transpose([0, 2, 1, 3])
                    nc.vector.tensor_mul(og[:, :, :, 2:4], og[:, :, :, 2:4], am_a)
                    # xy: sigmoid*stride + grid*stride
                    gs_a = gs_t[0:m, t0:t0 + ntl].transpose([0, 2, 1, 3])
                    og_xy = og[:, :, :, 0:2]
                    nc.vector.tensor_scalar_mul(og_xy, og_xy, float(stride))
                    nc.vector.tensor_add(og_xy, og_xy, gs_a)

                    # emit store chunks whose tile range is now fully decoded
                    while (next_chunk < len(chunks)
                           and chunks[next_chunk][0] + chunks[next_chunk][1]
                           <= t0 + ntl):
                        s0, snt = chunks[next_chunk]
                        nc.gpsimd.dma_start(
                            out=dram[f"y{l}"][:, :, s0:s0 + snt, :],
                            in_=ot[:, :, s0:s0 + snt, :])
                        next_chunk += 1
                assert next_chunk == len(chunks)
    nc.compile()
    return nc


_PROGS = {}


def _get_prog(use_bias: bool):
    if use_bias not in _PROGS:
        _PROGS[use_bias] = _build_program(use_bias)
    return _PROGS[use_bias]


def _host_gsam():
    """Merged [gs0|am0|gs1|am1|gs2|am2] host tensor, (128, 804) fp32."""
    cols = []
    for L in LEVELS:
        HW, W, stride = L["HW"], L["W"], L["stride"]
        nt = _ntiles(HW)
        hw = np.arange(nt * 128)
        gx = (hw % W).astype(np.float32) * stride
        gy = (hw // W).astype(np.float32) * stride
        gx[HW:] = 0.0
        gy[HW:] = 0.0
        gs = np.zeros((128, nt, NA, 2), np.float32)
        gs[:, :, :, 0] = gx.reshape(nt, 128).T[:, :, None]
        gs[:, :, :, 1] = gy.reshape(nt, 128).T[:, :, None]
        am = np.zeros((128, nt, NA, 2), np.float32)
        am[:, :, :, :] = np.asarray(L["anchors"], np.float32)[None, None, :, :]
        cols.append(gs.reshape(128, nt * 6))
        cols.append(am.reshape(128, nt * 6))
    return np.ascontiguousarray(
        np.concatenate(cols, axis=1).astype(np.float16))


_CONSTS = None


def _make_in_maps(xs, ws, bs, use_bias):
    global _CONSTS
    if _CONSTS is None:
        _CONSTS = _host_gsam()
    wts, xps = [], []
    for x, w, L in zip(xs, ws, LEVELS):
        KC = L["C"] // 128
        HW = L["HW"]
        # (C, NCOL) -> (128, KC*NCOL): row p col (k*NCOL+o) = w[o, k*128+p]
        wts.append(np.ascontiguousarray(
            w.T.astype(np.float16).reshape(KC, 128, NCOL)
            .transpose(1, 0, 2).reshape(128, KC * NCOL)))
        # (B, C, H, W) -> (B, 128, KC*HW): row p col (k*HW+hw) = x[k*128+p, hw]
        xps.append(np.ascontiguousarray(
            x.reshape(NCORES, KC, 128, HW).astype(np.float16)
            .transpose(0, 2, 1, 3).reshape(NCORES, 128, KC * HW)))
    in_maps = []
    for core in range(NCORES):
        im = {"gsam": _CONSTS}
        for l in range(len(LEVELS)):
            im[f"x{l}"] = xps[l][core]
            im[f"wt{l}"] = wts[l]
            if use_bias:
                im[f"b{l}"] = np.ascontiguousarray(
                    bs[l].reshape(1, NCOL).astype(np.float32))
        in_maps.append(im)
    return in_maps


def _assemble(results):
    """results[core][f"y{l}"] (128, NA, R, 89) -> (NCORES, 25200, 89) fp32."""
    out = np.empty((NCORES, 25200, NO), np.float32)
    for core in range(NCORES):
        parts = []
        for l, L in enumerate(LEVELS):
            HW = L["HW"]
            nt = _ntiles(HW)
            y = results[core][f"y{l}"].astype(np.float32)
            y = y.transpose(1, 2, 0, 3).reshape(NA, nt * 128, NO)[:, :HW, :]
            parts.append(y.reshape(NA * HW, NO))
        out[core] = np.concatenate(parts, axis=0)
    return out


def _run(x0, x1, x2, w0, b0, w1, b1, w2, b2, **spmd_kwargs):
    xs = [np.asarray(x, dtype=np.float32) for x in (x0, x1, x2)]
    ws = [np.asarray(w, dtype=np.float32) for w in (w0, w1, w2)]
    bs = [np.asarray(b, dtype=np.float32) for b in (b0, b1, b2)]
    use_bias = any(np.any(b != 0) for b in bs)
    in_maps = _make_in_maps(xs, ws, bs, use_bias)
    res = run_bass_kernel_spmd(_get_prog(use_bias), in_maps,
                               core_ids=list(range(NCORES)), **spmd_kwargs)
    return _assemble(res.results), res


def kernel(x0, x1, x2, w0, b0, w1, b1, w2, b2):
    out, _ = _run(x0, x1, x2, w0, b0, w1, b1, w2, b2)
    return out


def kernel_traced(x0, x1, x2, w0, b0, w1, b1, w2, b2):
    """Like kernel() but with NTFF tracing; returns (out, BassKernelResults)."""
    return _run(x0, x1, x2, w0, b0, w1, b1, w2, b2, trace=True)



# revision 4
# speedup vs baseline: 1.1215x; 1.1215x over previous
"""YOLO-detect head (1x1 conv + box decode) on 8 Trainium2 NeuronCores.

Data-parallel over batch: core b processes batch element b.

Per core, per level l (C channels, HW = ny*nx positions):
  p[hw, o] = sum_c x[c, hw] * w[o, c]      (o = a*89 + ch, a anchor, ch channel)
computed on the tensor engine as out = lhsT.T @ rhs with
  lhsT = x chunk  [K channels, M<=128 hw]   (stationary)
  rhs  = w.T chunk [K channels, N=267]      (moving)
so the PSUM result is already [hw, 267] - no on-chip transpose.
Level 0 runs in fp8(e4m3) DoubleRow mode: K=256 contracted per instruction
(w0 host-prescaled by 16 to clear e4m3's subnormal range; compensated by the
activation input scale 1/16). Levels 1-2 stay fp16.

Decode (what changed vs the tanh-trick version): the ACT engine uses the
sigmoid table DIRECTLY, so no post-activation affine pass on the DVE at all.
Per 4-tile PSUM group:
  ACT_b: fp8 sigmoid of all 89 cols -> resident fp8 output tile (also the
         store payload for the 85 "rest" cols; fp8 quantization of values in
         (0,1) is ~0.03 abs, far inside tolerance)
  ACT_a: fp32 sigmoid of the 2 wh cols only (needs precision)
  DVE:   u = 1-s      (tensor_scalar)
         q = s/u      (tensor_tensor divide)  == exp(p)
         wh = q*anchor (tensor_tensor)
         xy = (s8*stride) + grid*stride (one scalar_tensor_tensor, s8 from
              the fp8 tile; 0.03*stride abs error is negligible)
The exp table is never needed -> exactly one ACT table load total.

DMA regime:
  * inputs host-permuted so each level's x / w loads are 1-2 large
    fully-contiguous-per-partition DMAs on nc.sync (HWDGE)
  * outputs per level: y89 (128, NA, nt, 89) fp8 + y4 (128, NA, nt, 4) fp16;
    host transposes back and takes rest from y89, xy/wh from y4.
  * level-0 store chunks go through nc.gpsimd (SWDGE, otherwise-idle queue);
    level-1/2 stores go on nc.sync, whose load queue has drained by then.
"""

import numpy as np
import ml_dtypes

import concourse.bacc as bacc
import concourse.mybir as mybir
import concourse.tile as tile
from concourse.bass_utils import run_bass_kernel_spmd

F32 = mybir.dt.float32
F16 = mybir.dt.float16
F8 = mybir.dt.float8e4
AF = mybir.ActivationFunctionType
ALU = mybir.AluOpType
NP_F8 = ml_dtypes.float8_e4m3fn

NCORES = 8
NA = 3          # anchors per level
NO = 89         # channels per anchor (80 classes + 5 + 4)
NCOL = NA * NO  # 267
GROUP = 4       # full 128-row hw tiles per PSUM group (4 banks; 2 bufs = all 8)
W0SCALE = 16.0  # host pre-scale on w0 (fp8 subnormal avoidance)

LEVELS = [
    dict(C=256,  W=80, HW=6400, stride=8.0,
         anchors=((10.0, 13.0), (16.0, 30.0), (33.0, 23.0))),
    dict(C=512,  W=40, HW=1600, stride=16.0,
         anchors=((30.0, 61.0), (62.0, 45.0), (59.0, 119.0))),
    dict(C=1024, W=20, HW=400,  stride=32.0,
         anchors=((116.0, 90.0), (156.0, 198.0), (373.0, 326.0))),
]
NT = [(L["HW"] + 127) // 128 for L in LEVELS]   # 50, 13, 4
NTSUM = sum(NT)                                  # 67
LOFF = [sum(NT[:l]) for l in range(3)]           # tile offset of level l in gat

ORDER = (0, 1, 2)


def _groups(HW):
    """[(t0, n_full_tiles, rows)] with trailing partial tile as its own group."""
    full, rem = divmod(HW, 128)
    out = []
    t0 = 0
    while t0 < full:
        n = min(GROUP, full - t0)
        out.append((t0, n, 128))
        t0 += n
    if rem:
        out.append((full, 1, rem))
    return out


def _store_chunks(nt):
    """~16-tile store chunks aligned to group boundaries."""
    if nt <= 4 * GROUP + GROUP:
        return [(0, nt)]
    chunks = []
    s = 0
    while s < nt:
        e = min(s + 4 * GROUP, nt)
        if nt - e <= GROUP:
            e = nt
        chunks.append((s, e - s))
        s = e
    return chunks


def _build_program(use_bias: bool):
    # Bacc (not raw Bass): its compile() runs move_matmul_waits_to_ldweights +
    # generate_event_semaphores, without which walrus rejects instructions
    # that carry more than one semaphore wait.
    nc = bacc.Bacc("TRN2", target_bir_lowering=False, debug=False)

    dram = {}
    dram["x0"] = nc.dram_tensor("x0", (128, 2, LEVELS[0]["HW"]), F8,
                                kind="ExternalInput").ap()
    dram["wt0"] = nc.dram_tensor("wt0", (128, 2 * NCOL), F8,
                                 kind="ExternalInput").ap()
    for l in (1, 2):
        KC = LEVELS[l]["C"] // 128
        dram[f"x{l}"] = nc.dram_tensor(f"x{l}", (128, KC * LEVELS[l]["HW"]),
                                       F16, kind="ExternalInput").ap()
        dram[f"wt{l}"] = nc.dram_tensor(f"wt{l}", (128, KC * NCOL), F16,
                                        kind="ExternalInput").ap()
    for l in range(3):
        nt = NT[l]
        dram[f"y89_{l}"] = nc.dram_tensor(f"y89_{l}", (128, NA, nt, NO), F8,
                                          kind="ExternalOutput").ap()
        dram[f"y4_{l}"] = nc.dram_tensor(f"y4_{l}", (128, NA, nt, 4), F16,
                                         kind="ExternalOutput").ap()
        if use_bias:
            dram[f"b{l}"] = nc.dram_tensor(f"b{l}", (1, NCOL), F32,
                                           kind="ExternalInput").ap()
    # gat[p, t, a, 0:2] = grid*stride for hw row t*128+p (replicated over a)
    # gat[p, t, a, 2:4] = anchor wh (replicated over t)
    dram["gat"] = nc.dram_tensor("gat", (128, NTSUM, NA, 4), F16,
                                 kind="ExternalInput").ap()

    with tile.TileContext(nc) as tc:
        with tc.tile_pool(name="consts", bufs=1) as cpool, \
             tc.tile_pool(name="xbuf", bufs=1) as xpool, \
             tc.tile_pool(name="obuf", bufs=1) as opool, \
             tc.tile_pool(name="scr", bufs=2) as spool, \
             tc.tile_pool(name="ps", bufs=2, space="PSUM") as pspool:

            ones_t = None
            if use_bias:
                ones_t = cpool.tile([1, 128], F16, tag="ones", name="ones")
                nc.vector.memset(ones_t[:, :], 1.0)

            # ---- Phase A: all loads (nc.sync ring carries loads only) ----
            lvl = {}
            for l in ORDER:
                L = LEVELS[l]
                C, HW = L["C"], L["HW"]
                if l == 0:
                    wt_t = cpool.tile([128, 2 * NCOL], F8, tag="wt0",
                                      name="wt0sb")
                    nc.sync.dma_start(out=wt_t[:, :], in_=dram["wt0"][:, :])
                    xk = xpool.tile([128, 2, HW], F8, tag="x0", name="xk0")
                    # column-piece DMAs so level-0 matmuls start earlier
                    for (c0, c1) in ((0, 2048), (2048, 4224), (4224, HW)):
                        nc.sync.dma_start(out=xk[:, :, c0:c1],
                                          in_=dram["x0"][:, :, c0:c1])
                else:
                    KC = C // 128
                    wt_t = cpool.tile([128, KC * NCOL], F16, tag=f"wt{l}",
                                      name=f"wt{l}sb")
                    nc.sync.dma_start(out=wt_t[:, :], in_=dram[f"wt{l}"][:, :])
                    xk = xpool.tile([128, KC * HW], F16, tag=f"x{l}",
                                    name=f"xk{l}")
                    nc.sync.dma_start(out=xk[:, :], in_=dram[f"x{l}"][:, :])

                b_t = None
                if use_bias:
                    b_t = cpool.tile([1, NCOL], F32, tag=f"b{l}", name=f"bt{l}")
                    nc.gpsimd.dma_start(out=b_t[:, :], in_=dram[f"b{l}"][:, :])
                lvl[l] = dict(wt=wt_t, xk=xk, b_t=b_t)

                if l == ORDER[0]:
                    gat_t = cpool.tile([128, NTSUM, NA, 4], F16, tag="gat",
                                       name="gatsb")
                    nc.sync.dma_start(out=gat_t[:, :, :, :],
                                      in_=dram["gat"][:, :, :, :])

            # ---- Phase B: compute; level-0 stores via SWDGE (gpsimd) ----
            for l in ORDER:
                L = LEVELS[l]
                C, HW, stride = L["C"], L["HW"], L["stride"]
                KC = C // 128
                nt = NT[l]
                wt_t, xk, b_t = lvl[l]["wt"], lvl[l]["xk"], lvl[l]["b_t"]
                ascale = (1.0 / W0SCALE) if l == 0 else 1.0

                # whole level's decoded output stays resident; partition p
                # element (a, t, c) is output row hw = t*128+p of anchor a
                o89 = opool.tile([128, NA, nt, NO], F8, tag=f"o89_{l}",
                                 name=f"o89_{l}")
                o4 = opool.tile([128, NA, nt, 4], F16, tag=f"o4_{l}",
                                name=f"o4_{l}")

                chunks = _store_chunks(nt)
                next_chunk = 0

                for (t0, ntl, m) in _groups(HW):
                    ps = pspool.tile([128, GROUP, 512], F32, tag="ps",
                                     name=f"ps{l}_{t0}")
                    psf = ps.rearrange("p g x -> p (g x)")
                    for i in range(ntl):
                        t = t0 + i
                        if l == 0:
                            nc.tensor.matmul(
                                psf[0:m, i * 512:i * 512 + NCOL],
                                lhsT=xk[:, :, t * 128:t * 128 + m],
                                rhs=wt_t[:, :].rearrange("p (j o) -> p j o",
                                                         j=2),
                                start=True,
                                stop=not use_bias,
                                perf_mode=mybir.MatmulPerfMode.DoubleRow,
                            )
                        else:
                            for kc in range(KC):
                                nc.tensor.matmul(
                                    psf[0:m, i * 512:i * 512 + NCOL],
                                    lhsT=xk[:, kc * HW + t * 128:
                                            kc * HW + t * 128 + m],
                                    rhs=wt_t[:, kc * NCOL:(kc + 1) * NCOL],
                                    start=(kc == 0),
                                    stop=(kc == KC - 1 and not use_bias),
                                )
                        if use_bias:
                            nc.tensor.matmul(
                                psf[0:m, i * 512:i * 512 + NCOL],
                                lhsT=ones_t[:, 0:m],
                                rhs=b_t[:, :],
                                start=False,
                                stop=True,
                            )

                    # psum viewed (g, a, c)
                    ps_a = ps[0:m, 0:ntl, 0:NCOL].rearrange(
                        "p g (a c) -> p g a c", a=NA)
                    # output views enumerated (g, a, c) to match
                    o89v = o89[0:m, :, t0:t0 + ntl, :].transpose([0, 2, 1, 3])
                    o4v = o4[0:m, :, t0:t0 + ntl, :].transpose([0, 2, 1, 3])

                    # fp8 sigmoid of everything (rest payload + xy source)
                    nc.scalar.activation(o89v, ps_a, AF.Sigmoid, scale=ascale)
                    # fp32 sigmoid of the wh cols
                    s2 = spool.tile([128, GROUP, NA, 2], F32, tag="s2",
                                    name=f"s2_{l}_{t0}")
                    s2v = s2[0:m, 0:ntl]
                    nc.scalar.activation(s2v, ps_a[:, :, :, 2:4], AF.Sigmoid,
                                         scale=ascale)

                    # u = 1 - s ; q = s/u = exp(p) ; wh = q * anchor
                    u = spool.tile([128, GROUP, NA, 2], F32, tag="u",
                                   name=f"u_{l}_{t0}")
                    uv = u[0:m, 0:ntl]
                    nc.vector.tensor_scalar(uv, s2v, -1.0, 1.0, ALU.mult,
                                            ALU.add)
                    # no hw divide op on DVE: q = s * (1/u)
                    uf = u[0:m, 0:ntl].rearrange("p g a c -> p (g a c)")
                    nc.vector.reciprocal(uf, uf)
                    q = spool.tile([128, GROUP, NA, 2], F32, tag="q",
                                   name=f"q_{l}_{t0}")
                    qv = q[0:m, 0:ntl]
                    nc.vector.tensor_mul(qv, s2v, uv)
                    gat_g = gat_t[0:m, LOFF[l] + t0:LOFF[l] + t0 + ntl]
                    nc.vector.tensor_tensor(o4v[:, :, :, 2:4], qv,
                                            gat_g[:, :, :, 2:4], ALU.mult)
                    # xy = s8*stride + grid*stride (walrus caps STT at 3D, so
                    # two 4D-capable ops instead of one fused one)
                    nc.vector.tensor_scalar_mul(o4v[:, :, :, 0:2],
                                                o89v[:, :, :, 0:2],
                                                float(stride))
                    nc.vector.tensor_add(o4v[:, :, :, 0:2], o4v[:, :, :, 0:2],
                                         gat_g[:, :, :, 0:2])

                    # emit store chunks whose tile range is fully decoded
                    while (next_chunk < len(chunks)
                           and chunks[next_chunk][0] + chunks[next_chunk][1]
                           <= t0 + ntl):
                        s0, snt = chunks[next_chunk]
                        eng = nc.gpsimd if l == 0 else nc.sync
                        eng.dma_start(
                            out=dram[f"y89_{l}"][:, :, s0:s0 + snt, :],
                            in_=o89[:, :, s0:s0 + snt, :])
                        eng.dma_start(
                            out=dram[f"y4_{l}"][:, :, s0:s0 + snt, :],
                            in_=o4[:, :, s0:s0 + snt, :])
                        next_chunk += 1
                assert next_chunk == len(chunks)
    nc.compile()
    return nc


_PROGS = {}


def _get_prog(use_bias: bool):
    if use_bias not in _PROGS:
        _PROGS[use_bias] = _build_program(use_bias)
    return _PROGS[use_bias]


def _host_gat():
    """(128, 67, NA, 4) fp16: [...,0:2]=grid*stride, [...,2:4]=anchors."""
    gat = np.zeros((128, NTSUM, NA, 4), np.float32)
    for l, L in enumerate(LEVELS):
        HW, W, stride = L["HW"], L["W"], L["stride"]
        nt = NT[l]
        hw = np.arange(nt * 128)
        gx = (hw % W).astype(np.float32) * stride
        gy = (hw // W).astype(np.float32) * stride
        gx[HW:] = 0.0
        gy[HW:] = 0.0
        sl = gat[:, LOFF[l]:LOFF[l] + nt]
        sl[:, :, :, 0] = gx.reshape(nt, 128).T[:, :, None]
        sl[:, :, :, 1] = gy.reshape(nt, 128).T[:, :, None]
        sl[:, :, :, 2:4] = np.asarray(L["anchors"], np.float32)[None, None]
    return np.ascontiguousarray(gat.astype(np.float16))


_CONSTS = None


def _make_in_maps(xs, ws, bs, use_bias):
    global _CONSTS
    if _CONSTS is None:
        _CONSTS = _host_gat()
    wts, xps = [], []
    for l, (x, w, L) in enumerate(zip(xs, ws, LEVELS)):
        KC = L["C"] // 128
        HW = L["HW"]
        if l == 0:
            # w0: (267, 256) -> (128, 2*267) fp8, row p col (j*267+o) =
            # 16*w0[o, j*128+p]
            wts.append(np.ascontiguousarray(
                (w.T * W0SCALE).astype(NP_F8).reshape(2, 128, NCOL)
                .transpose(1, 0, 2).reshape(128, 2 * NCOL)))
            # x0: (B, 256, HW) -> (B, 128, 2, HW) fp8: [p, j, hw] = x[j*128+p]
            xps.append(np.ascontiguousarray(
                x.reshape(NCORES, 2, 128, HW).astype(NP_F8)
                .transpose(0, 2, 1, 3)))
        else:
            wts.append(np.ascontiguousarray(
                w.T.astype(np.float16).reshape(KC, 128, NCOL)
                .transpose(1, 0, 2).reshape(128, KC * NCOL)))
            xps.append(np.ascontiguousarray(
                x.reshape(NCORES, KC, 128, HW).astype(np.float16)
                .transpose(0, 2, 1, 3).reshape(NCORES, 128, KC * HW)))
    in_maps = []
    for core in range(NCORES):
        im = {"gat": _CONSTS}
        for l in range(len(LEVELS)):
            im[f"x{l}"] = xps[l][core]
            im[f"wt{l}"] = wts[l]
            if use_bias:
                scale = W0SCALE if l == 0 else 1.0
                im[f"b{l}"] = np.ascontiguousarray(
                    (bs[l] * scale).reshape(1, NCOL).astype(np.float32))
        in_maps.append(im)
    return in_maps


def _assemble(results):
    """y89 (128,NA,nt,89) fp8 + y4 (128,NA,nt,4) fp16 -> (8, 25200, 89)."""
    out = np.empty((NCORES, 25200, NO), np.float32)
    for core in range(NCORES):
        parts = []
        for l, L in enumerate(LEVELS):
            HW = L["HW"]
            nt = NT[l]
            y89 = results[core][f"y89_{l}"].astype(np.float32)
            y4 = results[core][f"y4_{l}"].astype(np.float32)
            y = y89.transpose(1, 2, 0, 3).reshape(NA, nt * 128, NO)[:, :HW, :]
            y4t = y4.transpose(1, 2, 0, 3).reshape(NA, nt * 128, 4)[:, :HW, :]
            y[:, :, 0:4] = y4t
            parts.append(y.reshape(NA * HW, NO))
        out[core] = np.concatenate(parts, axis=0)
    return out


def _run(x0, x1, x2, w0, b0, w1, b1, w2, b2, **spmd_kwargs):
    xs = [np.asarray(x, dtype=np.float32) for x in (x0, x1, x2)]
    ws = [np.asarray(w, dtype=np.float32) for w in (w0, w1, w2)]
    bs = [np.asarray(b, dtype=np.float32) for b in (b0, b1, b2)]
    use_bias = any(np.any(b != 0) for b in bs)
    in_maps = _make_in_maps(xs, ws, bs, use_bias)
    res = run_bass_kernel_spmd(_get_prog(use_bias), in_maps,
                               core_ids=list(range(NCORES)), **spmd_kwargs)
    return _assemble(res.results), res


def kernel(x0, x1, x2, w0, b0, w1, b1, w2, b2):
    out, _ = _run(x0, x1, x2, w0, b0, w1, b1, w2, b2)
    return out


def kernel_traced(x0, x1, x2, w0, b0, w1, b1, w2, b2):
    """Like kernel() but with NTFF tracing; returns (out, BassKernelResults)."""
    return _run(x0, x1, x2, w0, b0, w1, b1, w2, b2, trace=True)


# revision 7
# speedup vs baseline: 1.3826x; 1.2329x over previous
"""YOLO-detect head (1x1 conv + box decode) on 8 Trainium2 NeuronCores.

Data-parallel over batch: core b processes batch element b.

Per core, per level l (C channels, HW = ny*nx positions):
  p[hw, o] = sum_c x[c, hw] * w[o, c]      (o = a*89 + ch, a anchor, ch channel)
computed on the tensor engine as out = lhsT.T @ rhs with
  lhsT = x chunk  [K channels, M<=128 hw]   (stationary)
  rhs  = w.T chunk [K channels, N=267]      (moving)
so the PSUM result is already [hw, 267] - no on-chip transpose.
Level 0 runs in fp8(e4m3) DoubleRow mode: K=256 contracted per instruction
(w0 host-prescaled by 16 to clear e4m3's subnormal range; compensated by the
activation input scale 1/16). Levels 1-2 stay fp16.

Decode (what changed vs the tanh-trick version): the ACT engine uses the
sigmoid table DIRECTLY, so no post-activation affine pass on the DVE at all.
Per 4-tile PSUM group:
  ACT_b: fp8 sigmoid of all 89 cols -> resident fp8 output tile (also the
         store payload for the 85 "rest" cols; fp8 quantization of values in
         (0,1) is ~0.03 abs, far inside tolerance)
  ACT_a: fp32 sigmoid of the 2 wh cols only (needs precision)
  DVE:   u = 1-s      (tensor_scalar)
         q = s/u      (tensor_tensor divide)  == exp(p)
         wh = q*anchor (tensor_tensor)
         xy = (s8*stride) + grid*stride (one scalar_tensor_tensor, s8 from
              the fp8 tile; 0.03*stride abs error is negligible)
The exp table is never needed -> exactly one ACT table load total.

DMA regime:
  * inputs host-permuted so each level's x / w loads are 1-2 large
    fully-contiguous-per-partition DMAs on nc.sync (HWDGE)
  * outputs per level: y89 (128, NA, nt, 89) fp8 + y4 (128, NA, nt, 4) fp16;
    host transposes back and takes rest from y89, xy/wh from y4.
  * level-0 store chunks go through nc.gpsimd (SWDGE, otherwise-idle queue);
    level-1/2 stores go on nc.sync, whose load queue has drained by then.
"""

import numpy as np
import ml_dtypes

import concourse.bacc as bacc
import concourse.mybir as mybir
import concourse.tile as tile
from concourse.bass_utils import run_bass_kernel_spmd

F32 = mybir.dt.float32
F16 = mybir.dt.float16
F8 = mybir.dt.float8e4
AF = mybir.ActivationFunctionType
ALU = mybir.AluOpType
NP_F8 = ml_dtypes.float8_e4m3fn

NCORES = 8
NA = 3          # anchors per level
NO = 89         # channels per anchor (80 classes + 5 + 4)
NCOL = NA * NO  # 267
GROUP = 4       # full 128-row hw tiles per PSUM group (4 banks; 2 bufs = all 8)
W0SCALE = 16.0  # host pre-scale on w0 (fp8 subnormal avoidance)

LEVELS = [
    dict(C=256,  W=80, HW=6400, stride=8.0,
         anchors=((10.0, 13.0), (16.0, 30.0), (33.0, 23.0))),
    dict(C=512,  W=40, HW=1600, stride=16.0,
         anchors=((30.0, 61.0), (62.0, 45.0), (59.0, 119.0))),
    dict(C=1024, W=20, HW=400,  stride=32.0,
         anchors=((116.0, 90.0), (156.0, 198.0), (373.0, 326.0))),
]
NT = [(L["HW"] + 127) // 128 for L in LEVELS]   # 50, 13, 4
NTSUM = sum(NT)                                  # 67
LOFF = [sum(NT[:l]) for l in range(3)]           # tile offset of level l in gat

ORDER = (0, 1, 2)


def _groups(HW):
    """[(t0, n_full_tiles, rows)] with trailing partial tile as its own group."""
    full, rem = divmod(HW, 128)
    out = []
    t0 = 0
    while t0 < full:
        n = min(GROUP, full - t0)
        out.append((t0, n, 128))
        t0 += n
    if rem:
        out.append((full, 1, rem))
    return out


def _store_chunks(nt):
    """~16-tile store chunks aligned to group boundaries."""
    if nt <= 4 * GROUP + GROUP:
        return [(0, nt)]
    chunks = []
    s = 0
    while s < nt:
        e = min(s + 4 * GROUP, nt)
        if nt - e <= GROUP:
            e = nt
        chunks.append((s, e - s))
        s = e
    return chunks


def _build_program(use_bias: bool):
    # Bacc (not raw Bass): its compile() runs move_matmul_waits_to_ldweights +
    # generate_event_semaphores, without which walrus rejects instructions
    # that carry more than one semaphore wait.
    nc = bacc.Bacc("TRN2", target_bir_lowering=False, debug=False)

    dram = {}
    dram["x0"] = nc.dram_tensor("x0", (128, 2, LEVELS[0]["HW"]), F8,
                                kind="ExternalInput").ap()
    dram["wt0"] = nc.dram_tensor("wt0", (128, 2 * NCOL), F8,
                                 kind="ExternalInput").ap()
    for l in (1, 2):
        KC = LEVELS[l]["C"] // 128
        dram[f"x{l}"] = nc.dram_tensor(f"x{l}", (128, KC * LEVELS[l]["HW"]),
                                       F16, kind="ExternalInput").ap()
        dram[f"wt{l}"] = nc.dram_tensor(f"wt{l}", (128, KC * NCOL), F16,
                                        kind="ExternalInput").ap()
    for l in range(3):
        nt = NT[l]
        dram[f"y89_{l}"] = nc.dram_tensor(f"y89_{l}", (128, NA, nt, NO), F16,
                                          kind="ExternalOutput").ap()
        dram[f"y4_{l}"] = nc.dram_tensor(f"y4_{l}", (128, NA, nt, 4), F16,
                                         kind="ExternalOutput").ap()
        if use_bias:
            dram[f"b{l}"] = nc.dram_tensor(f"b{l}", (1, NCOL), F32,
                                           kind="ExternalInput").ap()
    # gat[p, t, a, 0:2] = grid*stride for hw row t*128+p (replicated over a)
    # gat[p, t, a, 2:4] = anchor wh (replicated over t)
    dram["gat"] = nc.dram_tensor("gat", (128, NTSUM, NA, 4), F16,
                                 kind="ExternalInput").ap()

    with tile.TileContext(nc) as tc:
        with tc.tile_pool(name="consts", bufs=1) as cpool, \
             tc.tile_pool(name="xbuf", bufs=1) as xpool, \
             tc.tile_pool(name="obuf", bufs=1) as opool, \
             tc.tile_pool(name="scr", bufs=2) as spool, \
             tc.tile_pool(name="ps", bufs=2, space="PSUM") as pspool:

            ones_t = None
            if use_bias:
                ones_t = cpool.tile([1, 128], F16, tag="ones", name="ones")
                nc.vector.memset(ones_t[:, :], 1.0)

            # ---- Phase A: all loads (nc.sync ring carries loads only) ----
            lvl = {}
            for l in ORDER:
                L = LEVELS[l]
                C, HW = L["C"], L["HW"]
                if l == 0:
                    wt_t = cpool.tile([128, 2 * NCOL], F8, tag="wt0",
                                      name="wt0sb")
                    nc.sync.dma_start(out=wt_t[:, :], in_=dram["wt0"][:, :])
                    xk = xpool.tile([128, 2, HW], F8, tag="x0", name="xk0")
                    # column-piece DMAs so level-0 matmuls start earlier
                    for (c0, c1) in ((0, 2048), (2048, 4224), (4224, HW)):
                        nc.sync.dma_start(out=xk[:, :, c0:c1],
                                          in_=dram["x0"][:, :, c0:c1])
                else:
                    KC = C // 128
                    wt_t = cpool.tile([128, KC * NCOL], F16, tag=f"wt{l}",
                                      name=f"wt{l}sb")
                    nc.sync.dma_start(out=wt_t[:, :], in_=dram[f"wt{l}"][:, :])
                    xk = xpool.tile([128, KC * HW], F16, tag=f"x{l}",
                                    name=f"xk{l}")
                    nc.sync.dma_start(out=xk[:, :], in_=dram[f"x{l}"][:, :])

                b_t = None
                if use_bias:
                    b_t = cpool.tile([1, NCOL], F32, tag=f"b{l}", name=f"bt{l}")
                    nc.gpsimd.dma_start(out=b_t[:, :], in_=dram[f"b{l}"][:, :])
                lvl[l] = dict(wt=wt_t, xk=xk, b_t=b_t)

                if l == ORDER[0]:
                    gat_t = cpool.tile([128, NTSUM, NA, 4], F16, tag="gat",
                                       name="gatsb")
                    nc.sync.dma_start(out=gat_t[:, :, :, :],
                                      in_=dram["gat"][:, :, :, :])

            # ---- Phase B: compute; level-0 stores via SWDGE (gpsimd) ----
            for l in ORDER:
                L = LEVELS[l]
                C, HW, stride = L["C"], L["HW"], L["stride"]
                KC = C // 128
                nt = NT[l]
                wt_t, xk, b_t = lvl[l]["wt"], lvl[l]["xk"], lvl[l]["b_t"]
                ascale = (1.0 / W0SCALE) if l == 0 else 1.0

                # whole level's decoded output stays resident; partition p
                # element (a, t, c) is output row hw = t*128+p of anchor a
                o89 = opool.tile([128, NA, nt, NO], F16, tag=f"o89_{l}",
                                 name=f"o89_{l}")
                o4 = opool.tile([128, NA, nt, 4], F16, tag=f"o4_{l}",
                                name=f"o4_{l}")

                chunks = _store_chunks(nt)
                next_chunk = 0

                for (t0, ntl, m) in _groups(HW):
                    ps = pspool.tile([128, GROUP, 512], F32, tag="ps",
                                     name=f"ps{l}_{t0}")
                    psf = ps.rearrange("p g x -> p (g x)")
                    for i in range(ntl):
                        t = t0 + i
                        if l == 0:
                            nc.tensor.matmul(
                                psf[0:m, i * 512:i * 512 + NCOL],
                                lhsT=xk[:, :, t * 128:t * 128 + m],
                                rhs=wt_t[:, :].rearrange("p (j o) -> p j o",
                                                         j=2),
                                start=True,
                                stop=not use_bias,
                                perf_mode=mybir.MatmulPerfMode.DoubleRow,
                            )
                        else:
                            for kc in range(KC):
                                nc.tensor.matmul(
                                    psf[0:m, i * 512:i * 512 + NCOL],
                                    lhsT=xk[:, kc * HW + t * 128:
                                            kc * HW + t * 128 + m],
                                    rhs=wt_t[:, kc * NCOL:(kc + 1) * NCOL],
                                    start=(kc == 0),
                                    stop=(kc == KC - 1 and not use_bias),
                                )
                        if use_bias:
                            nc.tensor.matmul(
                                psf[0:m, i * 512:i * 512 + NCOL],
                                lhsT=ones_t[:, 0:m],
                                rhs=b_t[:, :],
                                start=False,
                                stop=True,
                            )

                    # psum viewed (g, a, c)
                    ps_a = ps[0:m, 0:ntl, 0:NCOL].rearrange(
                        "p g (a c) -> p g a c", a=NA)
                    # output views enumerated (g, a, c) to match
                    o89v = o89[0:m, :, t0:t0 + ntl, :].transpose([0, 2, 1, 3])
                    o4v = o4[0:m, :, t0:t0 + ntl, :].transpose([0, 2, 1, 3])

                    # fp16 sigmoid of everything: the ONLY psum reader, so the
                    # psum group frees as early as possible (pipeline depth 2)
                    nc.scalar.activation(o89v, ps_a, AF.Sigmoid, scale=ascale)
                    s2v = o89v[:, :, :, 2:4]  # fp16 sigmoid of the wh cols

                    # u = 1 - s ; q = s/u = exp(p) ; wh = q * anchor
                    u = spool.tile([128, GROUP, NA, 2], F32, tag="u",
                                   name=f"u_{l}_{t0}")
                    uv = u[0:m, 0:ntl]
                    nc.vector.tensor_scalar(uv, s2v, -1.0, 1.0, ALU.mult,
                                            ALU.add)
                    # no hw divide op on DVE: q = s * (1/u)
                    uf = u[0:m, 0:ntl].rearrange("p g a c -> p (g a c)")
                    nc.vector.reciprocal_approx_fast(uf, uf)
                    q = spool.tile([128, GROUP, NA, 2], F32, tag="q",
                                   name=f"q_{l}_{t0}")
                    qv = q[0:m, 0:ntl]
                    nc.vector.tensor_mul(qv, s2v, uv)
                    gat_g = gat_t[0:m, LOFF[l] + t0:LOFF[l] + t0 + ntl]
                    nc.vector.tensor_tensor(o4v[:, :, :, 2:4], qv,
                                            gat_g[:, :, :, 2:4], ALU.mult)
                    # xy = s8*stride + grid*stride (walrus caps STT at 3D, so
                    # two 4D-capable ops instead of one fused one)
                    nc.vector.tensor_scalar_mul(o4v[:, :, :, 0:2],
                                                o89v[:, :, :, 0:2],
                                                float(stride))
                    nc.vector.tensor_add(o4v[:, :, :, 0:2], o4v[:, :, :, 0:2],
                                         gat_g[:, :, :, 0:2])

                    # emit store chunks whose tile range is fully decoded
                    while (next_chunk < len(chunks)
                           and chunks[next_chunk][0] + chunks[next_chunk][1]
                           <= t0 + ntl):
                        s0, snt = chunks[next_chunk]
                        eng = nc.gpsimd if l == 0 else nc.sync
                        eng.dma_start(
                            out=dram[f"y89_{l}"][:, :, s0:s0 + snt, :],
                            in_=o89[:, :, s0:s0 + snt, :])
                        eng.dma_start(
                            out=dram[f"y4_{l}"][:, :, s0:s0 + snt, :],
                            in_=o4[:, :, s0:s0 + snt, :])
                        next_chunk += 1
                assert next_chunk == len(chunks)
    nc.compile()
    return nc


_PROGS = {}


def _get_prog(use_bias: bool):
    if use_bias not in _PROGS:
        _PROGS[use_bias] = _build_program(use_bias)
    return _PROGS[use_bias]


def _host_gat():
    """(128, 67, NA, 4) fp16: [...,0:2]=grid*stride, [...,2:4]=anchors."""
    gat = np.zeros((128, NTSUM, NA, 4), np.float32)
    for l, L in enumerate(LEVELS):
        HW, W, stride = L["HW"], L["W"], L["stride"]
        nt = NT[l]
        hw = np.arange(nt * 128)
        gx = (hw % W).astype(np.float32) * stride
        gy = (hw // W).astype(np.float32) * stride
        gx[HW:] = 0.0
        gy[HW:] = 0.0
        sl = gat[:, LOFF[l]:LOFF[l] + nt]
        sl[:, :, :, 0] = gx.reshape(nt, 128).T[:, :, None]
        sl[:, :, :, 1] = gy.reshape(nt, 128).T[:, :, None]
        sl[:, :, :, 2:4] = np.asarray(L["anchors"], np.float32)[None, None]
    return np.ascontiguousarray(gat.astype(np.float16))


_CONSTS = None


def _make_in_maps(xs, ws, bs, use_bias):
    global _CONSTS
    if _CONSTS is None:
        _CONSTS = _host_gat()
    wts, xps = [], []
    for l, (x, w, L) in enumerate(zip(xs, ws, LEVELS)):
        KC = L["C"] // 128
        HW = L["HW"]
        if l == 0:
            # w0: (267, 256) -> (128, 2*267) fp8, row p col (j*267+o) =
            # 16*w0[o, j*128+p]
            wts.append(np.ascontiguousarray(
                (w.T * W0SCALE).astype(NP_F8).reshape(2, 128, NCOL)
                .transpose(1, 0, 2).reshape(128, 2 * NCOL)))
            # x0: (B, 256, HW) -> (B, 128, 2, HW) fp8: [p, j, hw] = x[j*128+p]
            xps.append(np.ascontiguousarray(
                x.reshape(NCORES, 2, 128, HW).astype(NP_F8)
                .transpose(0, 2, 1, 3)))
        else:
            wts.append(np.ascontiguousarray(
                w.T.astype(np.float16).reshape(KC, 128, NCOL)
                .transpose(1, 0, 2).reshape(128, KC * NCOL)))
            xps.append(np.ascontiguousarray(
                x.reshape(NCORES, KC, 128, HW).astype(np.float16)
                .transpose(0, 2, 1, 3).reshape(NCORES, 128, KC * HW)))
    in_maps = []
    for core in range(NCORES):
        im = {"gat": _CONSTS}
        for l in range(len(LEVELS)):
            im[f"x{l}"] = xps[l][core]
            im[f"wt{l}"] = wts[l]
            if use_bias:
                scale = W0SCALE if l == 0 else 1.0
                im[f"b{l}"] = np.ascontiguousarray(
                    (bs[l] * scale).reshape(1, NCOL).astype(np.float32))
        in_maps.append(im)
    return in_maps


def _assemble(results):
    """y89 (128,NA,nt,89) fp8 + y4 (128,NA,nt,4) fp16 -> (8, 25200, 89)."""
    out = np.empty((NCORES, 25200, NO), np.float32)
    for core in range(NCORES):
        parts = []
        for l, L in enumerate(LEVELS):
            HW = L["HW"]
            nt = NT[l]
            y89 = results[core][f"y89_{l}"].astype(np.float32)
            y4 = results[core][f"y4_{l}"].astype(np.float32)
            y = y89.transpose(1, 2, 0, 3).reshape(NA, nt * 128, NO)[:, :HW, :]
            y4t = y4.transpose(1, 2, 0, 3).reshape(NA, nt * 128, 4)[:, :HW, :]
            y[:, :, 0:4] = y4t
            parts.append(y.reshape(NA * HW, NO))
        out[core] = np.concatenate(parts, axis=0)
    return out


def _run(x0, x1, x2, w0, b0, w1, b1, w2, b2, **spmd_kwargs):
    xs = [np.asarray(x, dtype=np.float32) for x in (x0, x1, x2)]
    ws = [np.asarray(w, dtype=np.float32) for w in (w0, w1, w2)]
    bs = [np.asarray(b, dtype=np.float32) for b in (b0, b1, b2)]
    use_bias = any(np.any(b != 0) for b in bs)
    in_maps = _make_in_maps(xs, ws, bs, use_bias)
    res = run_bass_kernel_spmd(_get_prog(use_bias), in_maps,
                               core_ids=list(range(NCORES)), **spmd_kwargs)
    return _assemble(res.results), res


def kernel(x0, x1, x2, w0, b0, w1, b1, w2, b2):
    out, _ = _run(x0, x1, x2, w0, b0, w1, b1, w2, b2)
    return out


def kernel_traced(x0, x1, x2, w0, b0, w1, b1, w2, b2):
    """Like kernel() but with NTFF tracing; returns (out, BassKernelResults)."""
    return _run(x0, x1, x2, w0, b0, w1, b1, w2, b2, trace=True)


# revision 11
# speedup vs baseline: 1.3970x; 1.0104x over previous
"""YOLO-detect head (1x1 conv + box decode) on 8 Trainium2 NeuronCores.

Data-parallel over batch: core b processes batch element b.

Per core, per level l (C channels, HW = ny*nx positions):
  p[hw, o] = sum_c x[c, hw] * w[o, c]      (o = a*89 + ch, a anchor, ch channel)
computed on the tensor engine as out = lhsT.T @ rhs with
  lhsT = x chunk  [K channels, M<=128 hw]   (stationary)
  rhs  = w.T chunk [K channels, N=267]      (moving)
so the PSUM result is already [hw, 267] - no on-chip transpose.
Level 0 runs in fp8(e4m3) DoubleRow mode: K=256 contracted per instruction
(w0 host-prescaled by 16 to clear e4m3's subnormal range; compensated by the
activation input scale 1/16). Levels 1-2 stay fp16.

Decode (what changed vs the tanh-trick version): the ACT engine uses the
sigmoid table DIRECTLY, so no post-activation affine pass on the DVE at all.
Per 4-tile PSUM group:
  ACT_b: fp8 sigmoid of all 89 cols -> resident fp8 output tile (also the
         store payload for the 85 "rest" cols; fp8 quantization of values in
         (0,1) is ~0.03 abs, far inside tolerance)
  ACT_a: fp32 sigmoid of the 2 wh cols only (needs precision)
  DVE:   u = 1-s      (tensor_scalar)
         q = s/u      (tensor_tensor divide)  == exp(p)
         wh = q*anchor (tensor_tensor)
         xy = (s8*stride) + grid*stride (one scalar_tensor_tensor, s8 from
              the fp8 tile; 0.03*stride abs error is negligible)
The exp table is never needed -> exactly one ACT table load total.

DMA regime:
  * inputs host-permuted so each level's x / w loads are 1-2 large
    fully-contiguous-per-partition DMAs on nc.sync (HWDGE)
  * outputs per level: y89 (128, NA, nt, 89) fp8 + y4 (128, NA, nt, 4) fp16;
    host transposes back and takes rest from y89, xy/wh from y4.
  * level-0 store chunks go through nc.gpsimd (SWDGE, otherwise-idle queue);
    level-1/2 stores go on nc.sync, whose load queue has drained by then.
"""

import numpy as np
import ml_dtypes

import concourse.bacc as bacc
import concourse.mybir as mybir
import concourse.tile as tile
from concourse.bass_utils import run_bass_kernel_spmd

F32 = mybir.dt.float32
F16 = mybir.dt.float16
F8 = mybir.dt.float8e4
AF = mybir.ActivationFunctionType
ALU = mybir.AluOpType
NP_F8 = ml_dtypes.float8_e4m3fn

NCORES = 8
NA = 3          # anchors per level
NO = 89         # channels per anchor (80 classes + 5 + 4)
NCOL = NA * NO  # 267
GROUP = 4       # full 128-row hw tiles per PSUM group (4 banks; 2 bufs = all 8)
W0SCALE = 16.0  # host pre-scale on w0 (fp8 subnormal avoidance)

LEVELS = [
    dict(C=256,  W=80, HW=6400, stride=8.0,
         anchors=((10.0, 13.0), (16.0, 30.0), (33.0, 23.0))),
    dict(C=512,  W=40, HW=1600, stride=16.0,
         anchors=((30.0, 61.0), (62.0, 45.0), (59.0, 119.0))),
    dict(C=1024, W=20, HW=400,  stride=32.0,
         anchors=((116.0, 90.0), (156.0, 198.0), (373.0, 326.0))),
]
NT = [(L["HW"] + 127) // 128 for L in LEVELS]   # 50, 13, 4
NTSUM = sum(NT)                                  # 67
LOFF = [sum(NT[:l]) for l in range(3)]           # tile offset of level l in gat

ORDER = (0, 1, 2)


def _groups(HW):
    """[(t0, n_full_tiles, rows)] with trailing partial tile as its own group."""
    full, rem = divmod(HW, 128)
    out = []
    t0 = 0
    while t0 < full:
        n = min(GROUP, full - t0)
        out.append((t0, n, 128))
        t0 += n
    if rem:
        out.append((full, 1, rem))
    return out


def _store_chunks(nt):
    """~16-tile store chunks aligned to group boundaries."""
    if nt <= 2 * GROUP:
        return [(0, nt)]
    chunks = []
    s = 0
    while s < nt:
        e = min(s + 4 * GROUP, nt)
        if nt - e <= GROUP:
            e = nt
        chunks.append((s, e - s))
        s = e
    return chunks


def _build_program(use_bias: bool):
    # Bacc (not raw Bass): its compile() runs move_matmul_waits_to_ldweights +
    # generate_event_semaphores, without which walrus rejects instructions
    # that carry more than one semaphore wait.
    nc = bacc.Bacc("TRN2", target_bir_lowering=False, debug=False)

    dram = {}
    dram["x0"] = nc.dram_tensor("x0", (128, 2, LEVELS[0]["HW"]), F8,
                                kind="ExternalInput").ap()
    dram["wt0"] = nc.dram_tensor("wt0", (128, 2 * NCOL), F8,
                                 kind="ExternalInput").ap()
    for l in (1, 2):
        KC = LEVELS[l]["C"] // 128
        dram[f"x{l}"] = nc.dram_tensor(f"x{l}", (128, KC * LEVELS[l]["HW"]),
                                       F16, kind="ExternalInput").ap()
        dram[f"wt{l}"] = nc.dram_tensor(f"wt{l}", (128, KC * NCOL), F16,
                                        kind="ExternalInput").ap()
    for l in range(3):
        nt = NT[l]
        dram[f"y89_{l}"] = nc.dram_tensor(f"y89_{l}", (128, NA, nt, NO), F16,
                                          kind="ExternalOutput").ap()
        dram[f"y4_{l}"] = nc.dram_tensor(f"y4_{l}", (128, NA, nt, 4), F16,
                                         kind="ExternalOutput").ap()
        if use_bias:
            dram[f"b{l}"] = nc.dram_tensor(f"b{l}", (1, NCOL), F32,
                                           kind="ExternalInput").ap()
    # gat[p, t, a, 0:2] = grid*stride for hw row t*128+p (replicated over a)
    # gat[p, t, a, 2:4] = anchor wh (replicated over t)
    dram["gat"] = nc.dram_tensor("gat", (128, NTSUM, NA, 4), F16,
                                 kind="ExternalInput").ap()

    with tile.TileContext(nc) as tc:
        with tc.tile_pool(name="consts", bufs=1) as cpool, \
             tc.tile_pool(name="xbuf", bufs=1) as xpool, \
             tc.tile_pool(name="obuf", bufs=1) as opool, \
             tc.tile_pool(name="scr", bufs=2) as spool, \
             tc.tile_pool(name="ps", bufs=2, space="PSUM") as pspool:

            ones_t = None
            if use_bias:
                ones_t = cpool.tile([1, 128], F16, tag="ones", name="ones")
                nc.vector.memset(ones_t[:, :], 1.0)

            # ---- Phase A: all loads (nc.sync ring carries loads only) ----
            lvl = {}
            for l in ORDER:
                L = LEVELS[l]
                C, HW = L["C"], L["HW"]
                if l == 0:
                    wt_t = cpool.tile([128, 2 * NCOL], F8, tag="wt0",
                                      name="wt0sb")
                    nc.sync.dma_start(out=wt_t[:, :], in_=dram["wt0"][:, :])
                    xk = xpool.tile([128, 2, HW], F8, tag="x0", name="xk0")
                    # column-piece DMAs so level-0 matmuls start earlier
                    for (c0, c1) in ((0, 1024), (1024, 3072), (3072, HW)):
                        nc.sync.dma_start(out=xk[:, :, c0:c1],
                                          in_=dram["x0"][:, :, c0:c1])
                else:
                    KC = C // 128
                    wt_t = cpool.tile([128, KC * NCOL], F16, tag=f"wt{l}",
                                      name=f"wt{l}sb")
                    nc.sync.dma_start(out=wt_t[:, :], in_=dram[f"wt{l}"][:, :])
                    xk = xpool.tile([128, KC * HW], F16, tag=f"x{l}",
                                    name=f"xk{l}")
                    nc.sync.dma_start(out=xk[:, :], in_=dram[f"x{l}"][:, :])

                b_t = None
                if use_bias:
                    b_t = cpool.tile([1, NCOL], F32, tag=f"b{l}", name=f"bt{l}")
                    nc.gpsimd.dma_start(out=b_t[:, :], in_=dram[f"b{l}"][:, :])
                lvl[l] = dict(wt=wt_t, xk=xk, b_t=b_t)

                if l == ORDER[0]:
                    gat_t = cpool.tile([128, NTSUM, NA, 4], F16, tag="gat",
                                       name="gatsb")
                    nc.sync.dma_start(out=gat_t[:, :, :, :],
                                      in_=dram["gat"][:, :, :, :])

            # ---- Phase B: compute; level-0 stores via SWDGE (gpsimd) ----
            for l in ORDER:
                L = LEVELS[l]
                C, HW, stride = L["C"], L["HW"], L["stride"]
                KC = C // 128
                nt = NT[l]
                wt_t, xk, b_t = lvl[l]["wt"], lvl[l]["xk"], lvl[l]["b_t"]
                ascale = (1.0 / W0SCALE) if l == 0 else 1.0

                # whole level's decoded output stays resident; partition p
                # element (a, t, c) is output row hw = t*128+p of anchor a
                o89 = opool.tile([128, NA, nt, NO], F16, tag=f"o89_{l}",
                                 name=f"o89_{l}")
                o4 = opool.tile([128, NA, nt, 4], F16, tag=f"o4_{l}",
                                name=f"o4_{l}")

                chunks = _store_chunks(nt)
                next_chunk = 0

                for (t0, ntl, m) in _groups(HW):
                    ps = pspool.tile([128, GROUP, 512], F32, tag="ps",
                                     name=f"ps{l}_{t0}")
                    psf = ps.rearrange("p g x -> p (g x)")
                    for i in range(ntl):
                        t = t0 + i
                        if l == 0:
                            nc.tensor.matmul(
                                psf[0:m, i * 512:i * 512 + NCOL],
                                lhsT=xk[:, :, t * 128:t * 128 + m],
                                rhs=wt_t[:, :].rearrange("p (j o) -> p j o",
                                                         j=2),
                                start=True,
                                stop=not use_bias,
                                perf_mode=mybir.MatmulPerfMode.DoubleRow,
                            )
                        else:
                            for kc in range(KC):
                                nc.tensor.matmul(
                                    psf[0:m, i * 512:i * 512 + NCOL],
                                    lhsT=xk[:, kc * HW + t * 128:
                                            kc * HW + t * 128 + m],
                                    rhs=wt_t[:, kc * NCOL:(kc + 1) * NCOL],
                                    start=(kc == 0),
                                    stop=(kc == KC - 1 and not use_bias),
                                )
                        if use_bias:
                            nc.tensor.matmul(
                                psf[0:m, i * 512:i * 512 + NCOL],
                                lhsT=ones_t[:, 0:m],
                                rhs=b_t[:, :],
                                start=False,
                                stop=True,
                            )

                    # psum viewed (g, a, c)
                    ps_a = ps[0:m, 0:ntl, 0:NCOL].rearrange(
                        "p g (a c) -> p g a c", a=NA)
                    # output view enumerated (g, a, c) to match
                    o89v = o89[0:m, :, t0:t0 + ntl, :].transpose([0, 2, 1, 3])

                    # fp16 sigmoid of everything: the ONLY psum reader, so the
                    # psum group frees as early as possible (pipeline depth 2)
                    nc.scalar.activation(o89v, ps_a, AF.Sigmoid, scale=ascale)

                    # xy/wh fixups + stores run per ~16-tile chunk (they read
                    # the resident sigmoid tile, not psum, so they need not
                    # sit in the psum pipeline); partial-tile rows beyond m
                    # hold garbage, are computed anyway, and host-discarded
                    while (next_chunk < len(chunks)
                           and chunks[next_chunk][0] + chunks[next_chunk][1]
                           <= t0 + ntl):
                        s0, snt = chunks[next_chunk]
                        gat_c = gat_t[:, LOFF[l] + s0:LOFF[l] + s0 + snt] \
                            .transpose([0, 2, 1, 3])       # (p, a, t, c)
                        s2c = o89[:, :, s0:s0 + snt, 2:4]  # fp16 sigmoid
                        # u = 1 - s ; q = s/u = exp(p) ; wh = q * anchor
                        u = spool.tile([128, NA, 4 * GROUP + GROUP, 2], F32,
                                       tag="u", name=f"u_{l}_{s0}")
                        uv = u[:, :, 0:snt]
                        nc.vector.tensor_scalar(uv, s2c, -1.0, 1.0, ALU.mult,
                                                ALU.add)
                        # whole tile (contiguous); tail beyond snt is garbage
                        uf = u.rearrange("p a t c -> p (a t c)")
                        nc.vector.reciprocal_approx_fast(uf, uf)
                        q = spool.tile([128, NA, 4 * GROUP + GROUP, 2], F32,
                                       tag="q", name=f"q_{l}_{s0}")
                        qv = q[:, :, 0:snt]
                        nc.vector.tensor_mul(qv, s2c, uv)
                        o4c = o4[:, :, s0:s0 + snt, :]
                        nc.vector.tensor_tensor(o4c[:, :, :, 2:4], qv,
                                                gat_c[:, :, :, 2:4], ALU.mult)
                        # xy = s*stride + grid*stride
                        nc.vector.tensor_scalar_mul(o4c[:, :, :, 0:2],
                                                    o89[:, :, s0:s0 + snt, 0:2],
                                                    float(stride))
                        nc.vector.tensor_add(o4c[:, :, :, 0:2],
                                             o4c[:, :, :, 0:2],
                                             gat_c[:, :, :, 0:2])

                        eng = nc.gpsimd if l == 0 else nc.sync
                        eng.dma_start(
                            out=dram[f"y89_{l}"][:, :, s0:s0 + snt, :],
                            in_=o89[:, :, s0:s0 + snt, :])
                        eng.dma_start(
                            out=dram[f"y4_{l}"][:, :, s0:s0 + snt, :],
                            in_=o4[:, :, s0:s0 + snt, :])
                        next_chunk += 1
                assert next_chunk == len(chunks)
    nc.compile()
    return nc


_PROGS = {}


def _get_prog(use_bias: bool):
    if use_bias not in _PROGS:
        _PROGS[use_bias] = _build_program(use_bias)
    return _PROGS[use_bias]


def _host_gat():
    """(128, 67, NA, 4) fp16: [...,0:2]=grid*stride, [...,2:4]=anchors."""
    gat = np.zeros((128, NTSUM, NA, 4), np.float32)
    for l, L in enumerate(LEVELS):
        HW, W, stride = L["HW"], L["W"], L["stride"]
        nt = NT[l]
        hw = np.arange(nt * 128)
        gx = (hw % W).astype(np.float32) * stride
        gy = (hw // W).astype(np.float32) * stride
        gx[HW:] = 0.0
        gy[HW:] = 0.0
        sl = gat[:, LOFF[l]:LOFF[l] + nt]
        sl[:, :, :, 0] = gx.reshape(nt, 128).T[:, :, None]
        sl[:, :, :, 1] = gy.reshape(nt, 128).T[:, :, None]
        sl[:, :, :, 2:4] = np.asarray(L["anchors"], np.float32)[None, None]
    return np.ascontiguousarray(gat.astype(np.float16))


_CONSTS = None


def _make_in_maps(xs, ws, bs, use_bias):
    global _CONSTS
    if _CONSTS is None:
        _CONSTS = _host_gat()
    wts, xps = [], []
    for l, (x, w, L) in enumerate(zip(xs, ws, LEVELS)):
        KC = L["C"] // 128
        HW = L["HW"]
        if l == 0:
            # w0: (267, 256) -> (128, 2*267) fp8, row p col (j*267+o) =
            # 16*w0[o, j*128+p]
            wts.append(np.ascontiguousarray(
                (w.T * W0SCALE).astype(NP_F8).reshape(2, 128, NCOL)
                .transpose(1, 0, 2).reshape(128, 2 * NCOL)))
            # x0: (B, 256, HW) -> (B, 128, 2, HW) fp8: [p, j, hw] = x[j*128+p]
            xps.append(np.ascontiguousarray(
                x.reshape(NCORES, 2, 128, HW).astype(NP_F8)
                .transpose(0, 2, 1, 3)))
        else:
            wts.append(np.ascontiguousarray(
                w.T.astype(np.float16).reshape(KC, 128, NCOL)
                .transpose(1, 0, 2).reshape(128, KC * NCOL)))
            xps.append(np.ascontiguousarray(
                x.reshape(NCORES, KC, 128, HW).astype(np.float16)
                .transpose(0, 2, 1, 3).reshape(NCORES, 128, KC * HW)))
    in_maps = []
    for core in range(NCORES):
        im = {"gat": _CONSTS}
        for l in range(len(LEVELS)):
            im[f"x{l}"] = xps[l][core]
            im[f"wt{l}"] = wts[l]
            if use_bias:
                scale = W0SCALE if l == 0 else 1.0
                im[f"b{l}"] = np.ascontiguousarray(
                    (bs[l] * scale).reshape(1, NCOL).astype(np.float32))
        in_maps.append(im)
    return in_maps


def _assemble(results):
    """y89 (128,NA,nt,89) fp8 + y4 (128,NA,nt,4) fp16 -> (8, 25200, 89)."""
    out = np.empty((NCORES, 25200, NO), np.float32)
    for core in range(NCORES):
        parts = []
        for l, L in enumerate(LEVELS):
            HW = L["HW"]
            nt = NT[l]
            y89 = results[core][f"y89_{l}"].astype(np.float32)
            y4 = results[core][f"y4_{l}"].astype(np.float32)
            y = y89.transpose(1, 2, 0, 3).reshape(NA, nt * 128, NO)[:, :HW, :]
            y4t = y4.transpose(1, 2, 0, 3).reshape(NA, nt * 128, 4)[:, :HW, :]
            y[:, :, 0:4] = y4t
            parts.append(y.reshape(NA * HW, NO))
        out[core] = np.concatenate(parts, axis=0)
    return out


def _run(x0, x1, x2, w0, b0, w1, b1, w2, b2, **spmd_kwargs):
    xs = [np.asarray(x, dtype=np.float32) for x in (x0, x1, x2)]
    ws = [np.asarray(w, dtype=np.float32) for w in (w0, w1, w2)]
    bs = [np.asarray(b, dtype=np.float32) for b in (b0, b1, b2)]
    use_bias = any(np.any(b != 0) for b in bs)
    in_maps = _make_in_maps(xs, ws, bs, use_bias)
    res = run_bass_kernel_spmd(_get_prog(use_bias), in_maps,
                               core_ids=list(range(NCORES)), **spmd_kwargs)
    return _assemble(res.results), res


def kernel(x0, x1, x2, w0, b0, w1, b1, w2, b2):
    out, _ = _run(x0, x1, x2, w0, b0, w1, b1, w2, b2)
    return out


def kernel_traced(x0, x1, x2, w0, b0, w1, b1, w2, b2):
    """Like kernel() but with NTFF tracing; returns (out, BassKernelResults)."""
    return _run(x0, x1, x2, w0, b0, w1, b1, w2, b2, trace=True)
